# revision 20
# baseline (speedup 1.0000x reference)
"""BertSelfAttention (with relative-position bias + key-unary scores) on 8 trn2 cores.

Sharding: 8 cores = 2 batches x 4 query-blocks of 256 rows.
Host prep: transpose hidden/qkv_weight (cast bf16); precompute rel-pos bias
(bf16, [h,k,q] layout) and per-k unary+mask (fp32) per core.
Device per core: bf16 GEMM inputs, fp32 PSUM/softmax/ctx.
  kT [f,seq] bf16, v [seq,f] fp32 (ones-augmented), qT [f,qblk] bf16
  per head: scoresT[k,q] = kT.T-slice @ qT-slice (PSUM fp32)
            psum = 0.125*psum + relb (DVE STT, relb bf16)
            expT = Exp(psum + umk[k])   (ACT, per-partition fp32 bias)
            ctx[q, 65] = sum_k expT @ v_aug (col 64 = softmax denominator)
            out[:, h*64:+64] = ctx[:, :64] * 1/ctx[:, 64]
Every DMA writes a fresh (never-reused) SBUF target: walrus allows only one
sync-wait per DMA instruction, so DMA targets must be write-once.
"""

import numpy as np

B, S, H, NH, BINS = 2, 1024, 1024, 16, 32
D = H // NH          # 64
NCORES = 8
QB = B * S // NCORES  # 256 query rows per core
MASK_ADD = -40.0

_CACHED = {}


def _build_bass():
    import concourse.bass as bass
    import concourse.mybir as mybir
    from concourse.tile import TileContext

    fp32 = mybir.dt.float32
    bf16 = mybir.dt.bfloat16
    Alu = mybir.AluOpType
    Act = mybir.ActivationFunctionType

    nc = bass.Bass()
    MA = S + QB + 1
    # megaA: [H, 1024(xT) + 256(xqT) + 1(q_bias)] bf16
    megaA = nc.dram_tensor("megaA", (H, MA), bf16, kind="ExternalInput")
    wT = nc.dram_tensor("wT", (H, 3 * H), bf16, kind="ExternalInput")  # qkv_weight.T
    # rel-pos bias per head, [k, q] layout, bf16
    relb = [
        nc.dram_tensor(f"relb{g}", (8 * S, QB), bf16, kind="ExternalInput")
        for g in range(2)
    ]
    # unary + mask, per-k fp32: [k%128, k//128, h]
    umk = nc.dram_tensor("umk", (128, 8, NH), fp32, kind="ExternalInput")
    qb = nc.dram_tensor("qb", (128, 8), fp32, kind="ExternalInput")
    out0 = nc.dram_tensor("out0", (128, H), fp32, kind="ExternalOutput")
    out1 = nc.dram_tensor("out1", (128, H), fp32, kind="ExternalOutput")

    with TileContext(nc) as tc:
        with (
            tc.tile_pool(name="persist", bufs=1) as pp,
            tc.tile_pool(name="exp", bufs=1) as ep,
            tc.tile_pool(name="small", bufs=4) as sp,
            tc.tile_pool(name="ps_a", bufs=2, space="PSUM") as ps_a,
            tc.tile_pool(name="ps_sc", bufs=4, space="PSUM") as ps_sc,
            tc.tile_pool(name="ps_ctx", bufs=2, space="PSUM") as ps_ctx,
        ):
            # ---- resident SBUF tensors (all DMA targets write-once) ----
            ma_all = pp.tile([128, 8, MA], bf16, name="ma_all")
            w_all = pp.tile([128, 8, 3 * H], bf16, name="w_all")
            relb_all = [
                pp.tile([128, 64, QB], bf16, name=f"relb_t{g}") for g in range(2)
            ]
            umk_t = pp.tile([128, 8, NH], fp32, name="umk_t")
            qb_t = pp.tile([128, 8], fp32, name="qb_t")
            kt_all = pp.tile([128, 8, S], bf16, name="kt_all")
            v_all = pp.tile([128, 8, NH, D + 1], fp32, name="v_all")
            qt_all = pp.tile([128, 8, QB], bf16, name="qt_all")
            out_sb0 = pp.tile([128, H], fp32, name="out_sb0")
            out_sb1 = pp.tile([128, H], fp32, name="out_sb1")
            out_sb = [out_sb0, out_sb1]

            nc.scalar.dma_start(out=ma_all, in_=megaA.rearrange("(c p) s -> p c s", p=128))
            nc.scalar.dma_start(out=w_all, in_=wT.rearrange("(c p) n -> p c n", p=128))
            nc.scalar.dma_start(out=umk_t, in_=umk[:, :, :])
            nc.scalar.dma_start(out=qb_t, in_=qb[:, :])
            for g in range(2):
                nc.scalar.dma_start(
                    out=relb_all[g], in_=relb[g].rearrange("(a p) q -> p a q", p=128)
                )

            # absorber chains: each engine observes every input-DMA lane via
            # single-wait micro-ops (walrus: max one sync-wait per instruction)
            ps0 = ps_a.tile([128, 512], fp32, tag="ps_a", name="ps0")
            nc.tensor.matmul(
                ps0[0:1, 0:1], lhsT=ma_all[:, 0, 0:1], rhs=ma_all[:, 0, 0:1],
                start=True, stop=True,
            )
            nc.tensor.matmul(
                ps0[0:1, 0:1], lhsT=w_all[:, 0, 0:1], rhs=w_all[:, 0, 0:1],
                start=True, stop=True,
            )
            ab = sp.tile([1, 8], fp32, tag="ab", name="ab")
            nc.vector.tensor_copy(ab[0:1, 0:1], qb_t[0:1, 0:1])
            nc.vector.tensor_copy(ab[0:1, 1:2], relb_all[0][0:1, 0, 0:1])
            nc.vector.tensor_copy(ab[0:1, 2:3], relb_all[1][0:1, 0, 0:1])
            nc.vector.tensor_copy(ab[0:1, 3:4], ma_all[0:1, 0, 0:1])
            nc.vector.tensor_copy(ab[0:1, 4:5], w_all[0:1, 0, 0:1])
            ab2 = sp.tile([1, 8], fp32, tag="ab2", name="ab2")
            nc.scalar.copy(ab2[0:1, 0:1], umk_t[0:1, 0, 0:1])
            nc.scalar.copy(ab2[0:1, 1:2], ma_all[0:1, 0, 0:1])

            # ---- kT = Wk @ X^T  (out [f-tile, seq], bf16) ----
            for ft in range(8):
                for half in range(2):
                    ps = ps_a.tile([128, 512], fp32, tag="ps_a", name="ps_k")
                    for h8 in range(8):
                        nc.tensor.matmul(
                            ps,
                            lhsT=w_all[:, h8, H + ft * 128 : H + (ft + 1) * 128],
                            rhs=ma_all[:, h8, half * 512 : (half + 1) * 512],
                            start=(h8 == 0),
                            stop=(h8 == 7),
                        )
                    nc.vector.tensor_copy(
                        kt_all[:, ft, half * 512 : (half + 1) * 512], ps
                    )

            # ---- qT = Wq @ Xq^T + q_bias (bf16) ----
            for ft in range(8):
                ps = ps_sc.tile([128, QB], fp32, tag="ps_q", name="ps_q")
                for h8 in range(8):
                    nc.tensor.matmul(
                        ps,
                        lhsT=w_all[:, h8, ft * 128 : (ft + 1) * 128],
                        rhs=ma_all[:, h8, S : S + QB],
                        start=(h8 == 0),
                        stop=(h8 == 7),
                    )
                nc.vector.tensor_scalar(
                    out=qt_all[:, ft, :],
                    in0=ps,
                    scalar1=qb_t[:, ft : ft + 1],
                    scalar2=None,
                    op0=Alu.add,
                )

            # ---- v = X @ Wv^T (natural fp32), with ones column ----
            for st in range(8):
                for half in range(2):
                    ps = ps_a.tile([128, 512], fp32, tag="ps_a", name="ps_v")
                    for h8 in range(8):
                        nc.tensor.matmul(
                            ps,
                            lhsT=ma_all[:, h8, st * 128 : (st + 1) * 128],
                            rhs=w_all[:, h8, 2 * H + half * 512 : 2 * H + (half + 1) * 512],
                            start=(h8 == 0),
                            stop=(h8 == 7),
                        )
                    nc.vector.tensor_copy(
                        v_all[:, st, half * 8 : (half + 1) * 8, 0:D],
                        ps.rearrange("p (h d) -> p h d", d=D),
                    )
            nc.vector.memset(v_all[:, :, :, D : D + 1], 1.0)

            # ---- attention per head ----
            for h in range(NH):
                fth, r0 = h // 2, (h % 2) * 64
                exp_t = ep.tile([128, 8, QB], fp32, tag="exp", name="exp_t")
                if h > 0:
                    nc.scalar.copy(exp_t[0:1, 0, 0:1], exp_t[0:1, 0, 0:1])
                for kt in range(8):
                    ps = ps_sc.tile([128, QB], fp32, tag="ps_q", name="ps_s")
                    nc.tensor.matmul(
                        ps,
                        lhsT=kt_all[r0 : r0 + 64, fth, kt * 128 : (kt + 1) * 128],
                        rhs=qt_all[r0 : r0 + 64, fth, :],
                        start=True,
                        stop=True,
                    )
                    nc.vector.scalar_tensor_tensor(
                        out=ps,
                        in0=ps,
                        scalar=0.125,
                        in1=relb_all[h // 8][:, (h % 8) * 8 + kt, :],
                        op0=Alu.mult,
                        op1=Alu.add,
                    )
                    nc.scalar.activation(
                        exp_t[:, kt, :], ps, Act.Exp,
                        bias=umk_t[:, kt, h : h + 1],
                    )
                for qh in range(2):
                    psc = ps_ctx.tile([128, D + 1], fp32, tag="ps_ctx", name="ps_c")
                    for kt in range(8):
                        nc.tensor.matmul(
                            psc,
                            lhsT=exp_t[:, kt, qh * 128 : (qh + 1) * 128],
                            rhs=v_all[:, kt, h, :],
                            start=(kt == 0),
                            stop=(kt == 7),
                        )
                    rec = sp.tile([128, 1], fp32, tag="rec", name="rec")
                    nc.vector.reciprocal(rec, psc[:, D : D + 1])
                    nc.vector.tensor_scalar(
                        out=out_sb[qh][:, h * D : (h + 1) * D],
                        in0=psc[:, 0:D],
                        scalar1=rec,
                        scalar2=None,
                        op0=Alu.mult,
                    )

            nc.sync.dma_start(out=out0[:, :], in_=out_sb0)
            nc.sync.dma_start(out=out1[:, :], in_=out_sb1)
    _split_multiwaits(nc, mybir)
    return nc


def _split_multiwaits(nc, mybir):
    """This container's walrus accepts only ONE sync-wait per instruction.
    Hoist all but the last wait of any instruction onto standalone
    InstEventSemaphore ops spliced just before it on the same engine."""
    n = 0
    for bb in nc.m.functions[0].blocks:
        insts = bb.instructions
        new = []
        for inst in insts:
            si = inst.sync_info
            ow = list(si.on_wait) if si and si.on_wait else []
            if len(ow) > 1:
                for w in ow[:-1]:
                    ev = mybir.InstEventSemaphore(
                        name=f"EVW-{n}", ins=[], outs=[]
                    )
                    n += 1
                    try:
                        ev.engine = inst.engine
                    except Exception:
                        pass
                    ev.sync_info = mybir.SyncInfo(on_wait=[w], on_update=[])
                    new.append(ev)
                inst.sync_info = mybir.SyncInfo(
                    on_wait=[ow[-1]], on_update=list(si.on_update or [])
                )
            new.append(inst)
        bb.instructions[:] = new


def kernel(**inputs):
    import ml_dtypes

    bf = ml_dtypes.bfloat16
    hidden = np.ascontiguousarray(np.asarray(inputs["hidden_states"], dtype=np.float32))
    mask = np.asarray(inputs["attention_mask"]).reshape(B, S)
    qkv_w = np.asarray(inputs["qkv_weight"], dtype=np.float32)
    q_bias = np.asarray(inputs["q_bias"], dtype=np.float32).reshape(-1)
    v_bias = np.asarray(inputs["v_bias"], dtype=np.float32).reshape(-1)
    rel_pos = np.asarray(inputs["rel_pos"], dtype=np.float32)
    rel_w = np.asarray(inputs["rel_pos_weight"], dtype=np.float32)      # [16, 32]
    ku_w = np.asarray(inputs["key_unary_weight"], dtype=np.float32)[0, :, :, 0]  # [16, 1024]

    wT = np.ascontiguousarray(qkv_w.T).astype(bf)            # [1024, 3072]

    in_maps = []
    for c in range(NCORES):
        b, qi = divmod(c, 4)
        q0 = qi * QB
        megaA = np.empty((H, S + QB + 1), dtype=np.float32)
        megaA[:, :S] = hidden[b].T
        megaA[:, S : S + QB] = hidden[b, q0 : q0 + QB].T
        megaA[:, S + QB] = q_bias
        # rel bias [h, k, q] bf16 (small amplitude -> bf16 safe)
        t = rel_pos[b, q0 : q0 + QB].reshape(-1, BINS) @ rel_w.T   # [(q k), h]
        relbT = t.reshape(QB, S, NH).transpose(2, 1, 0)            # [h, k, q]
        # unary + mask, per-k fp32: umk[p, c, h] = unary[h, c*128+p] + madd[c*128+p]
        unary = ku_w @ hidden[b].T                                 # [h, k]
        madd = np.where(mask[b], MASK_ADD, 0.0).astype(np.float32) # [k]
        umk = np.ascontiguousarray(
            (unary + madd[None, :]).reshape(NH, 8, 128).transpose(2, 1, 0)
        )                                                           # [128, 8, 16]
        qbT = np.ascontiguousarray(q_bias.reshape(8, 128).T)  # [128, 8]
        im = dict(megaA=megaA.astype(bf), wT=wT, umk=umk, qb=qbT)
        relbT2 = np.ascontiguousarray(relbT.reshape(2, 8 * S, QB)).astype(bf)
        im["relb0"], im["relb1"] = relbT2[0], relbT2[1]
        in_maps.append(im)

    if "nc" not in _CACHED:
        _CACHED["nc"] = _build_bass()
    from concourse.bass_utils import run_bass_kernel_spmd

    res = run_bass_kernel_spmd(_CACHED["nc"], in_maps, core_ids=list(range(NCORES)))
    _CACHED["last_results"] = res

    out = np.empty((B, S, H), dtype=np.float32)
    for c in range(NCORES):
        b, qi = divmod(c, 4)
        out[b, qi * QB : qi * QB + 128] = res.results[c]["out0"]
        out[b, qi * QB + 128 : (qi + 1) * QB] = res.results[c]["out1"]
    out += v_bias[None, None, :]
    return out
